# revision 11
# baseline (speedup 1.0000x reference)
"""Trainium2 Bass kernel for nn_NormalComparisonModel (dense comparison MLP).

Model: p1 = mean_L(f1), p2 = mean_L(f2);
out[i,j] = sigmoid(gelu(gelu([p1_i, p2_j, p1_i-p2_j] @ W1 + b1) @ W2 + b2) @ W3 + b3)

Key algebraic restructure: with U = W1[0:D] + W1[2D:3D], V = W1[D:2D] - W1[2D:3D],
    [p1_i, p2_j, p1_i-p2_j] @ W1 = p1_i @ U + p2_j @ V
so layer 1 collapses from O(N1*N2*3D*DENSE) to O((N1+N2)*D*DENSE) flops.

Sharding: data-parallel over N1 (32 rows/core); f2 pooling sharded over N2 with an
AllGather of the pooled features (32KB/core) so no core reads more than its own
4MB slice of either input.

Layouts: A2T / x1pre / x1T are j-major ([p, j*KCH+kc]) and B1T is i-major
([p, i*KCH+kc]) so the per-i bias broadcast has a contiguous last AP dim --
that qualifies the DVE tensor_tensor add for the 2x bf16 perf mode.
"""
import numpy as np

N_CORES = 8
N1, N2, L, D = 256, 256, 128, 256
IPC = N1 // N_CORES   # i rows per core
JPC = N2 // N_CORES   # j rows per core (pooling shard)
DENSE = 1024
H2 = 512
KCH = DENSE // 128    # 8 k-chunks
CB = H2 // 128        # 4 c-blocks
DC = D // 128         # 2 d-chunks

_CACHED_NC = None


def build_kernel():
    import concourse.bacc as bacc
    import concourse.mybir as mybir
    import concourse.tile as tile
    from concourse.ap import AP as APcls

    f32 = mybir.dt.float32
    f32r = mybir.dt.float32r
    bf16 = mybir.dt.bfloat16
    AF = mybir.ActivationFunctionType

    def bcast_mid(ap2d, n):
        """[P, K] AP -> [P, n, K] AP with a step-0 middle dim."""
        lay = [list(d) for d in ap2d.ap]
        assert len(lay) == 2
        return APcls(ap2d.tensor, ap2d.offset, [lay[0], [0, n], lay[1]])

    nc = bacc.Bacc("TRN2", target_bir_lowering=False, debug=False,
                   num_devices=N_CORES)

    f1c = nc.declare_dram_parameter("f1c", [IPC, L, D], f32, isOutput=False)
    f2c = nc.declare_dram_parameter("f2c", [JPC, L, D], f32, isOutput=False)
    W1 = nc.declare_dram_parameter("W1", [3 * D, DENSE], f32, isOutput=False)
    b1 = nc.declare_dram_parameter("b1", [DENSE], f32, isOutput=False)
    W2 = nc.declare_dram_parameter("W2", [DENSE, H2], f32, isOutput=False)
    b2 = nc.declare_dram_parameter("b2", [H2], f32, isOutput=False)
    W3 = nc.declare_dram_parameter("W3", [H2, 1], f32, isOutput=False)
    b3r = nc.declare_dram_parameter("b3r", [IPC, 1], f32, isOutput=False)
    out_c = nc.declare_dram_parameter("out_c", [IPC, N2], f32, isOutput=True)

    with tile.TileContext(nc) as tc:
        with (
            tc.tile_pool(name="const", bufs=1) as cpool,
            tc.tile_pool(name="work", bufs=3) as wpool,
            tc.tile_pool(name="psum", bufs=2, space="PSUM") as pp,
            tc.tile_pool(name="dram", bufs=1, space="DRAM") as dpool,
        ):
            # ---------- pooling (f2 first: feeds the AllGather critical path) ----
            ones_f32 = cpool.tile([128, 2], f32, tag="ones_f32")
            nc.vector.memset(ones_f32[:], 1.0 / L)
            ones_t = cpool.tile([128, 2], f32r, tag="ones")
            nc.vector.tensor_copy(ones_t[:], ones_f32[:])

            # pT[d, row] = mean_l f[row, l, d]; f32r matmuls (f tile is the
            # stationary operand) accumulate columns into persistent psum tiles
            NLD = 4  # load chunks per source
            pool_sb = {}
            for (name, src, nrow) in (("p2", f2c, JPC), ("p1", f1c, IPC)):
                fall = cpool.tile([128, nrow * D], f32r, tag=f"fall_{name}")
                rows_per = nrow // NLD
                for c in range(NLD):
                    nc.gpsimd.dma_start(
                        out=fall[:, c * rows_per * D:(c + 1) * rows_per * D]
                            .rearrange("l (r d) -> l r d", r=rows_per),
                        in_=src[c * rows_per:(c + 1) * rows_per]
                            .rearrange("r l d -> l r d"))
                ps_dc = [pp.tile([128, 2 * nrow], f32, tag="small",
                                 name=f"ps_{name}_{dc}")
                         for dc in range(DC)]
                for r in range(nrow):
                    for dc in range(DC):
                        nc.tensor.matmul(
                            ps_dc[dc][:, 2 * r:2 * r + 2],
                            fall[:, r * D + dc * 128:r * D + (dc + 1) * 128],
                            ones_t[:], start=True, stop=True)
                pT = cpool.tile([128, DC * nrow], f32r, tag=f"{name}T")
                for dc in range(DC):
                    nc.vector.tensor_copy(pT[:, dc * nrow:(dc + 1) * nrow],
                                          ps_dc[dc][:, 0::2])
                pool_sb[name] = pT
            p2T = pool_sb["p2"]
            p1Tr = pool_sb["p1"]

            # ---------- AllGather p2T ----------
            p2loc = dpool.tile([DC, 128, JPC], f32, tag="p2loc")
            p2glob = dpool.tile([N_CORES, DC, 128, JPC], f32, tag="p2glob")
            nc.sync.dma_start(
                out=p2loc[:].rearrange("dc d j -> d dc j"),
                in_=p2T[:].bitcast(f32).rearrange("d (dc j) -> d dc j", dc=DC))
            nc.gpsimd.collective_compute(
                "AllGather",
                mybir.AluOpType.bypass,
                ins=[p2loc[:].opt()],
                outs=[p2glob[:].opt()],
                replica_groups=[list(range(N_CORES))],
            )
            p2all = []
            for dc in range(DC):
                t = cpool.tile([128, N2], f32r, tag=f"p2all_{dc}")
                nc.gpsimd.dma_start(
                    out=t[:].rearrange("d (c j) -> d c j", c=N_CORES),
                    in_=p2glob[:, dc, :, :].rearrange("c d j -> d c j"))
                p2all.append(t)

            # ---------- constants / weights ----------
            b1T = cpool.tile([128, KCH], f32, tag="b1T")     # b1T[p, kc] = b1[kc*128+p]
            nc.sync.dma_start(out=b1T[:], in_=b1[:].rearrange("(c p) -> p c", p=128))
            b2T = cpool.tile([128, CB], f32, tag="b2T")
            nc.sync.dma_start(out=b2T[:], in_=b2[:].rearrange("(c p) -> p c", p=128))
            w3T = cpool.tile([128, CB], bf16, tag="w3T")
            nc.gpsimd.dma_start(out=w3T[:],
                                in_=W3[:].rearrange("(c p) o -> p (c o)", p=128))
            b3t = cpool.tile([IPC, 1], f32, tag="b3t")
            nc.sync.dma_start(out=b3t[:], in_=b3r[:])

            # W2 in bf16, one mega cast DMA; w2m[:, kc*H2 + c]
            w2m = cpool.tile([128, KCH * H2], bf16, tag="w2m")
            nc.gpsimd.dma_start(
                out=w2m[:].rearrange("p (kc c) -> p kc c", kc=KCH),
                in_=W2[:].rearrange("(kc p) c -> p kc c", p=128))

            # U = W1a + W1c, V = W1b - W1c  (f32r, layout [d, k'])
            w1t = []
            for r in range(6):
                t = cpool.tile([128, DENSE], f32, tag=f"w1_{r}")
                nc.sync.dma_start(out=t[:], in_=W1[r * 128:(r + 1) * 128, :])
                w1t.append(t)
            Ut, Vt = [], []
            for dc in range(DC):
                u = cpool.tile([128, DENSE], f32r, tag=f"u_{dc}")
                v = cpool.tile([128, DENSE], f32r, tag=f"v_{dc}")
                nc.vector.tensor_add(u[:], w1t[dc][:], w1t[4 + dc][:])
                nc.vector.tensor_sub(v[:], w1t[2 + dc][:], w1t[4 + dc][:])
                Ut.append(u)
                Vt.append(v)

            # ---------- A1T (+b1 -> B1T, i-major), A2T (j-major) ----------
            B1T = cpool.tile([128, IPC * KCH], bf16, tag="B1T")  # col = i*KCH + kc
            A2T = cpool.tile([128, N2 * KCH], bf16, tag="A2T")   # col = j*KCH + kc
            for kb in range(KCH):
                pa = pp.tile([128, IPC], f32, tag="small")
                for dc in range(DC):
                    nc.tensor.matmul(pa[:], Ut[dc][:, kb * 128:(kb + 1) * 128],
                                     p1Tr[:, dc * IPC:(dc + 1) * IPC],
                                     start=(dc == 0), stop=(dc == DC - 1))
                nc.vector.tensor_scalar_add(B1T[:, kb::KCH], pa[:],
                                            b1T[:, kb:kb + 1])
            for kb in range(KCH):
                pa2 = pp.tile([128, N2], f32, tag="big")
                for dc in range(DC):
                    nc.tensor.matmul(pa2[:], Vt[dc][:, kb * 128:(kb + 1) * 128],
                                     p2all[dc][:], start=(dc == 0),
                                     stop=(dc == DC - 1))
                nc.vector.tensor_copy(A2T[:, kb::KCH], pa2[:])

            # ---------- main loop over i ----------
            outst = cpool.tile([IPC, N2], f32, tag="outst")
            for i in range(IPC):
                # x1pre = A2T + B1T[:, i-block] broadcast over j (2x bf16 DVE)
                x1pre = wpool.tile([128, N2 * KCH], bf16, tag="x1pre")
                nc.vector.tensor_add(
                    x1pre[:].rearrange("p (j k) -> p j k", k=KCH),
                    A2T[:].rearrange("p (j k) -> p j k", k=KCH),
                    bcast_mid(B1T[:, i * KCH:(i + 1) * KCH], N2))
                # x1 = gelu(x1pre) -> bf16
                x1T = wpool.tile([128, N2 * KCH], bf16, tag="x1T")
                nc.scalar.activation(x1T[:], x1pre[:], AF.Gelu)

                # layer 2: h2preT[cb] = sum_kc W2[kc,cb].T @ x1T[kc]
                ph2 = pp.tile([128, CB * N2], f32, tag="big")
                for cb in range(CB):
                    for kc in range(KCH):
                        nc.tensor.matmul(
                            ph2[:, cb * N2:(cb + 1) * N2],
                            w2m[:, kc * H2 + cb * 128:kc * H2 + (cb + 1) * 128],
                            x1T[:, kc::KCH],
                            start=(kc == 0), stop=(kc == KCH - 1))
                # h2b = ph2 + b2 (broadcast), then gelu -> bf16
                h2b = wpool.tile([128, CB * N2], f32, tag="h2b")
                nc.vector.tensor_add(
                    h2b[:].rearrange("p (c j) -> p c j", c=CB),
                    ph2[:].rearrange("p (c j) -> p c j", c=CB),
                    b2T[:].broadcast_to((128, CB, N2)))
                h2T = wpool.tile([128, CB * N2], bf16, tag="h2T")
                nc.scalar.activation(h2T[:], h2b[:], AF.Gelu)

                # layer 3: out_pre[1, N2] = sum_cb w3T[:, cb].T @ h2T[cb]
                pl3 = pp.tile([1, N2], f32, tag="small")
                for cb in range(CB):
                    nc.tensor.matmul(pl3[:], w3T[:, cb:cb + 1],
                                     h2T[:, cb * N2:(cb + 1) * N2],
                                     start=(cb == 0), stop=(cb == CB - 1))
                # collect row i (cross-partition move via small DMA)
                orow = wpool.tile([1, N2], f32, tag="orow")
                nc.vector.tensor_copy(orow[:], pl3[:])
                nc.sync.dma_start(out=outst[i:i + 1, :], in_=orow[:])

            # ---------- sigmoid + store ----------
            osg = cpool.tile([IPC, N2], f32, tag="osg")
            nc.scalar.activation(osg[:], outst[:], AF.Sigmoid, bias=b3t[:])
            nc.sync.dma_start(out=out_c[:], in_=osg[:])

    nc.finalize()
    return nc


def kernel(**inputs):
    from concourse.bass_utils import run_bass_kernel_spmd

    global _CACHED_NC
    f1 = np.ascontiguousarray(np.asarray(inputs["f1"], dtype=np.float32))
    f2 = np.ascontiguousarray(np.asarray(inputs["f2"], dtype=np.float32))
    W1 = np.ascontiguousarray(np.asarray(inputs["W1"], dtype=np.float32))
    b1 = np.asarray(inputs["b1"], dtype=np.float32)
    W2 = np.ascontiguousarray(np.asarray(inputs["W2"], dtype=np.float32))
    b2 = np.asarray(inputs["b2"], dtype=np.float32)
    W3 = np.ascontiguousarray(np.asarray(inputs["W3"], dtype=np.float32))
    b3 = np.asarray(inputs["b3"], dtype=np.float32)
    b3r = np.full((IPC, 1), b3.reshape(-1)[0], dtype=np.float32)

    if _CACHED_NC is None:
        _CACHED_NC = build_kernel()
    nc = _CACHED_NC

    in_maps = []
    for k in range(N_CORES):
        in_maps.append({
            "f1c": np.ascontiguousarray(f1[k * IPC:(k + 1) * IPC]),
            "f2c": np.ascontiguousarray(f2[k * JPC:(k + 1) * JPC]),
            "W1": W1, "b1": b1, "W2": W2, "b2": b2, "W3": W3, "b3r": b3r,
        })
    res = run_bass_kernel_spmd(nc, in_maps, core_ids=list(range(N_CORES)))
    out = np.concatenate([res.results[k]["out_c"] for k in range(N_CORES)],
                         axis=0)
    return out.astype(np.float32)


# revision 13
# speedup vs baseline: 1.9731x; 1.9731x over previous
"""Trainium2 Bass kernel for nn_NormalComparisonModel (dense comparison MLP).

Model: p1 = mean_L(f1), p2 = mean_L(f2);
out[i,j] = sigmoid(gelu(gelu([p1_i, p2_j, p1_i-p2_j] @ W1 + b1) @ W2 + b2) @ W3 + b3)

Key algebraic restructure: with U = W1[0:D] + W1[2D:3D], V = W1[D:2D] - W1[2D:3D],
    [p1_i, p2_j, p1_i-p2_j] @ W1 = p1_i @ U + p2_j @ V
so layer 1 collapses from O(N1*N2*3D*DENSE) to O((N1+N2)*D*DENSE) flops.

Sharding: data-parallel over N1 (32 rows/core); f2 pooling sharded over N2 with an
AllGather of the pooled features (32KB/core) so no core reads more than its own
4MB slice of either input.

Layouts: A2T / x1pre / x1T are j-major ([p, j*KCH+kc]) and B1T is i-major
([p, i*KCH+kc]) so the per-i bias broadcast has a contiguous last AP dim --
that qualifies the DVE tensor_tensor add for the 2x bf16 perf mode.
"""
import numpy as np

N_CORES = 8
N1, N2, L, D = 256, 256, 128, 256
IPC = N1 // N_CORES   # i rows per core
JPC = N2 // N_CORES   # j rows per core (pooling shard)
DENSE = 1024
H2 = 512
KCH = DENSE // 128    # 8 k-chunks
CB = H2 // 128        # 4 c-blocks
DC = D // 128         # 2 d-chunks

_CACHED_NC = None


def build_kernel():
    import concourse.bacc as bacc
    import concourse.mybir as mybir
    import concourse.tile as tile
    from concourse.ap import AP as APcls

    f32 = mybir.dt.float32
    f32r = mybir.dt.float32r
    bf16 = mybir.dt.bfloat16
    AF = mybir.ActivationFunctionType

    def bcast_mid(ap2d, n):
        """[P, K] AP -> [P, n, K] AP with a step-0 middle dim."""
        lay = [list(d) for d in ap2d.ap]
        assert len(lay) == 2
        return APcls(ap2d.tensor, ap2d.offset, [lay[0], [0, n], lay[1]])

    nc = bacc.Bacc("TRN2", target_bir_lowering=False, debug=False,
                   num_devices=N_CORES)

    f1c = nc.declare_dram_parameter("f1c", [IPC, L, D], f32, isOutput=False)
    f2c = nc.declare_dram_parameter("f2c", [JPC, L, D], f32, isOutput=False)
    W1 = nc.declare_dram_parameter("W1", [3 * D, DENSE], f32, isOutput=False)
    b1 = nc.declare_dram_parameter("b1", [DENSE], f32, isOutput=False)
    W2 = nc.declare_dram_parameter("W2", [DENSE, H2], f32, isOutput=False)
    b2 = nc.declare_dram_parameter("b2", [H2], f32, isOutput=False)
    W3 = nc.declare_dram_parameter("W3", [H2, 1], f32, isOutput=False)
    b3r = nc.declare_dram_parameter("b3r", [IPC, 1], f32, isOutput=False)
    out_c = nc.declare_dram_parameter("out_c", [IPC, N2], f32, isOutput=True)

    with tile.TileContext(nc) as tc:
        with (
            tc.tile_pool(name="const", bufs=1) as cpool,
            tc.tile_pool(name="work", bufs=3) as wpool,
            tc.tile_pool(name="psum", bufs=2, space="PSUM") as pp,
            tc.tile_pool(name="dram", bufs=1, space="DRAM") as dpool,
        ):
            # ---------- pooling (f2 first: feeds the AllGather critical path) ----
            ones_f32 = cpool.tile([128, 2], f32, tag="ones_f32")
            nc.vector.memset(ones_f32[:], 1.0 / L)
            ones_t = cpool.tile([128, 2], f32r, tag="ones")
            nc.vector.tensor_copy(ones_t[:], ones_f32[:])

            # pT[d, row] = mean_l f[row, l, d]; f32r matmuls (f tile is the
            # stationary operand) accumulate columns into persistent psum tiles
            NLD = 4  # load chunks per source
            pool_sb = {}
            for (name, src, nrow) in (("p2", f2c, JPC), ("p1", f1c, IPC)):
                fall = cpool.tile([128, nrow * D], f32r, tag=f"fall_{name}")
                rows_per = nrow // NLD
                for c in range(NLD):
                    nc.gpsimd.dma_start(
                        out=fall[:, c * rows_per * D:(c + 1) * rows_per * D]
                            .rearrange("l (r d) -> l r d", r=rows_per),
                        in_=src[c * rows_per:(c + 1) * rows_per]
                            .rearrange("r l d -> l r d"))
                ps_dc = [pp.tile([128, 2 * nrow], f32, tag="small",
                                 name=f"ps_{name}_{dc}")
                         for dc in range(DC)]
                for r in range(nrow):
                    for dc in range(DC):
                        nc.tensor.matmul(
                            ps_dc[dc][:, 2 * r:2 * r + 2],
                            fall[:, r * D + dc * 128:r * D + (dc + 1) * 128],
                            ones_t[:], start=True, stop=True)
                pT = cpool.tile([128, DC * nrow], f32r, tag=f"{name}T")
                for dc in range(DC):
                    nc.vector.tensor_copy(pT[:, dc * nrow:(dc + 1) * nrow],
                                          ps_dc[dc][:, 0::2])
                pool_sb[name] = pT
            p2T = pool_sb["p2"]
            p1Tr = pool_sb["p1"]

            # ---------- AllGather p2T ----------
            p2loc = dpool.tile([DC, 128, JPC], f32, tag="p2loc")
            p2glob = dpool.tile([N_CORES, DC, 128, JPC], f32, tag="p2glob")
            nc.sync.dma_start(
                out=p2loc[:].rearrange("dc d j -> d dc j"),
                in_=p2T[:].bitcast(f32).rearrange("d (dc j) -> d dc j", dc=DC))
            nc.gpsimd.collective_compute(
                "AllGather",
                mybir.AluOpType.bypass,
                ins=[p2loc[:].opt()],
                outs=[p2glob[:].opt()],
                replica_groups=[list(range(N_CORES))],
            )
            p2all = []
            for dc in range(DC):
                t = cpool.tile([128, N2], f32r, tag=f"p2all_{dc}")
                nc.gpsimd.dma_start(
                    out=t[:].rearrange("d (c j) -> d c j", c=N_CORES),
                    in_=p2glob[:, dc, :, :].rearrange("c d j -> d c j"))
                p2all.append(t)

            # ---------- constants / weights ----------
            b1T = cpool.tile([128, KCH], f32, tag="b1T")     # b1T[p, kc] = b1[kc*128+p]
            nc.sync.dma_start(out=b1T[:], in_=b1[:].rearrange("(c p) -> p c", p=128))
            b2T = cpool.tile([128, CB], f32, tag="b2T")
            nc.sync.dma_start(out=b2T[:], in_=b2[:].rearrange("(c p) -> p c", p=128))
            w3T = cpool.tile([128, CB], bf16, tag="w3T")
            nc.gpsimd.dma_start(out=w3T[:],
                                in_=W3[:].rearrange("(c p) o -> p (c o)", p=128))
            b3t = cpool.tile([IPC, 1], f32, tag="b3t")
            nc.sync.dma_start(out=b3t[:], in_=b3r[:])

            # W2 in bf16, one mega cast DMA; w2m[:, kc*H2 + c]
            w2m = cpool.tile([128, KCH * H2], bf16, tag="w2m")
            nc.gpsimd.dma_start(
                out=w2m[:].rearrange("p (kc c) -> p kc c", kc=KCH),
                in_=W2[:].rearrange("(kc p) c -> p kc c", p=128))

            # U = W1a + W1c, V = W1b - W1c  (f32r, layout [d, k'])
            w1t = []
            for r in range(6):
                t = cpool.tile([128, DENSE], f32, tag=f"w1_{r}")
                nc.sync.dma_start(out=t[:], in_=W1[r * 128:(r + 1) * 128, :])
                w1t.append(t)
            Ut, Vt = [], []
            for dc in range(DC):
                u = cpool.tile([128, DENSE], f32r, tag=f"u_{dc}")
                v = cpool.tile([128, DENSE], f32r, tag=f"v_{dc}")
                nc.vector.tensor_add(u[:], w1t[dc][:], w1t[4 + dc][:])
                nc.vector.tensor_sub(v[:], w1t[2 + dc][:], w1t[4 + dc][:])
                Ut.append(u)
                Vt.append(v)

            # ---------- A1T (+b1 -> B1T, i-major), A2T (j-major) ----------
            B1T = cpool.tile([128, IPC * KCH], bf16, tag="B1T")  # col = i*KCH + kc
            A2T = cpool.tile([128, N2 * KCH], bf16, tag="A2T")   # col = j*KCH + kc
            for kb in range(KCH):
                pa = pp.tile([128, IPC], f32, tag="small")
                for dc in range(DC):
                    nc.tensor.matmul(pa[:], Ut[dc][:, kb * 128:(kb + 1) * 128],
                                     p1Tr[:, dc * IPC:(dc + 1) * IPC],
                                     start=(dc == 0), stop=(dc == DC - 1))
                nc.vector.tensor_scalar_add(B1T[:, kb::KCH], pa[:],
                                            b1T[:, kb:kb + 1])
            for kb in range(KCH):
                pa2 = pp.tile([128, N2], f32, tag="big")
                for dc in range(DC):
                    nc.tensor.matmul(pa2[:], Vt[dc][:, kb * 128:(kb + 1) * 128],
                                     p2all[dc][:], start=(dc == 0),
                                     stop=(dc == DC - 1))
                nc.vector.tensor_copy(A2T[:, kb::KCH], pa2[:])

            # ---------- main loop over i ----------
            outst = cpool.tile([IPC, N2], f32, tag="outst")
            for i in range(IPC):
                # x1pre = A2T + B1T[:, i-block] broadcast over j (2x bf16 DVE)
                x1pre = wpool.tile([128, N2 * KCH], bf16, tag="x1pre")
                nc.vector.tensor_add(
                    x1pre[:].rearrange("p (j k) -> p j k", k=KCH),
                    A2T[:].rearrange("p (j k) -> p j k", k=KCH),
                    bcast_mid(B1T[:, i * KCH:(i + 1) * KCH], N2))
                # x1 = gelu(x1pre) -> bf16; re-layout j-major -> kc-major so the
                # matmul rhs below is contiguous (strided ACT input)
                x1T = wpool.tile([128, KCH * N2], bf16, tag="x1T")
                nc.scalar.activation(
                    x1T[:],
                    x1pre[:].rearrange("p (j k) -> p k j", k=KCH),
                    AF.Gelu)

                # layer 2: h2preT[cb] = sum_kc W2[kc,cb].T @ x1T[kc]
                ph2 = pp.tile([128, CB * N2], f32, tag="big")
                for cb in range(CB):
                    for kc in range(KCH):
                        nc.tensor.matmul(
                            ph2[:, cb * N2:(cb + 1) * N2],
                            w2m[:, kc * H2 + cb * 128:kc * H2 + (cb + 1) * 128],
                            x1T[:, kc * N2:(kc + 1) * N2],
                            start=(kc == 0), stop=(kc == KCH - 1))
                # h2b = ph2 + b2 (broadcast), then gelu -> bf16
                h2b = wpool.tile([128, CB * N2], f32, tag="h2b")
                nc.vector.tensor_add(
                    h2b[:].rearrange("p (c j) -> p c j", c=CB),
                    ph2[:].rearrange("p (c j) -> p c j", c=CB),
                    b2T[:].broadcast_to((128, CB, N2)))
                h2T = wpool.tile([128, CB * N2], bf16, tag="h2T")
                nc.scalar.activation(h2T[:], h2b[:], AF.Gelu)

                # layer 3: out_pre[1, N2] = sum_cb w3T[:, cb].T @ h2T[cb]
                pl3 = pp.tile([1, N2], f32, tag="small")
                for cb in range(CB):
                    nc.tensor.matmul(pl3[:], w3T[:, cb:cb + 1],
                                     h2T[:, cb * N2:(cb + 1) * N2],
                                     start=(cb == 0), stop=(cb == CB - 1))
                # collect row i (cross-partition move via small DMA)
                orow = wpool.tile([1, N2], f32, tag="orow")
                nc.vector.tensor_copy(orow[:], pl3[:])
                nc.sync.dma_start(out=outst[i:i + 1, :], in_=orow[:])

            # ---------- sigmoid + store ----------
            osg = cpool.tile([IPC, N2], f32, tag="osg")
            nc.scalar.activation(osg[:], outst[:], AF.Sigmoid, bias=b3t[:])
            nc.sync.dma_start(out=out_c[:], in_=osg[:])

    nc.finalize()
    return nc


def kernel(**inputs):
    from concourse.bass_utils import run_bass_kernel_spmd

    global _CACHED_NC
    f1 = np.ascontiguousarray(np.asarray(inputs["f1"], dtype=np.float32))
    f2 = np.ascontiguousarray(np.asarray(inputs["f2"], dtype=np.float32))
    W1 = np.ascontiguousarray(np.asarray(inputs["W1"], dtype=np.float32))
    b1 = np.asarray(inputs["b1"], dtype=np.float32)
    W2 = np.ascontiguousarray(np.asarray(inputs["W2"], dtype=np.float32))
    b2 = np.asarray(inputs["b2"], dtype=np.float32)
    W3 = np.ascontiguousarray(np.asarray(inputs["W3"], dtype=np.float32))
    b3 = np.asarray(inputs["b3"], dtype=np.float32)
    b3r = np.full((IPC, 1), b3.reshape(-1)[0], dtype=np.float32)

    if _CACHED_NC is None:
        _CACHED_NC = build_kernel()
    nc = _CACHED_NC

    in_maps = []
    for k in range(N_CORES):
        in_maps.append({
            "f1c": np.ascontiguousarray(f1[k * IPC:(k + 1) * IPC]),
            "f2c": np.ascontiguousarray(f2[k * JPC:(k + 1) * JPC]),
            "W1": W1, "b1": b1, "W2": W2, "b2": b2, "W3": W3, "b3r": b3r,
        })
    res = run_bass_kernel_spmd(nc, in_maps, core_ids=list(range(N_CORES)))
    out = np.concatenate([res.results[k]["out_c"] for k in range(N_CORES)],
                         axis=0)
    return out.astype(np.float32)


# revision 14
# speedup vs baseline: 2.3039x; 1.1677x over previous
"""Trainium2 Bass kernel for nn_NormalComparisonModel (dense comparison MLP).

Model: p1 = mean_L(f1), p2 = mean_L(f2);
out[i,j] = sigmoid(gelu(gelu([p1_i, p2_j, p1_i-p2_j] @ W1 + b1) @ W2 + b2) @ W3 + b3)

Key algebraic restructure: with U = W1[0:D] + W1[2D:3D], V = W1[D:2D] - W1[2D:3D],
    [p1_i, p2_j, p1_i-p2_j] @ W1 = p1_i @ U + p2_j @ V
so layer 1 collapses from O(N1*N2*3D*DENSE) to O((N1+N2)*D*DENSE) flops.

Sharding: data-parallel over N1 (32 rows/core); f2 pooling sharded over N2 with an
AllGather of the pooled features (32KB/core) so no core reads more than its own
4MB slice of either input.

Layouts: A2T / x1pre / x1T are j-major ([p, j*KCH+kc]) and B1T is i-major
([p, i*KCH+kc]) so the per-i bias broadcast has a contiguous last AP dim --
that qualifies the DVE tensor_tensor add for the 2x bf16 perf mode.
"""
import numpy as np

N_CORES = 8
N1, N2, L, D = 256, 256, 128, 256
IPC = N1 // N_CORES   # i rows per core
JPC = N2 // N_CORES   # j rows per core (pooling shard)
DENSE = 1024
H2 = 512
KCH = DENSE // 128    # 8 k-chunks
CB = H2 // 128        # 4 c-blocks
DC = D // 128         # 2 d-chunks

_CACHED_NC = None


def build_kernel():
    import concourse.bacc as bacc
    import concourse.mybir as mybir
    import concourse.tile as tile
    from concourse.ap import AP as APcls

    f32 = mybir.dt.float32
    f32r = mybir.dt.float32r
    bf16 = mybir.dt.bfloat16
    AF = mybir.ActivationFunctionType

    def bcast_mid(ap2d, n):
        """[P, K] AP -> [P, n, K] AP with a step-0 middle dim."""
        lay = [list(d) for d in ap2d.ap]
        assert len(lay) == 2
        return APcls(ap2d.tensor, ap2d.offset, [lay[0], [0, n], lay[1]])

    nc = bacc.Bacc("TRN2", target_bir_lowering=False, debug=False,
                   num_devices=N_CORES)

    f1c = nc.declare_dram_parameter("f1c", [IPC, L, D], f32, isOutput=False)
    f2c = nc.declare_dram_parameter("f2c", [JPC, L, D], f32, isOutput=False)
    W1 = nc.declare_dram_parameter("W1", [3 * D, DENSE], f32, isOutput=False)
    b1 = nc.declare_dram_parameter("b1", [DENSE], f32, isOutput=False)
    W2 = nc.declare_dram_parameter("W2", [DENSE, H2], f32, isOutput=False)
    b2 = nc.declare_dram_parameter("b2", [H2], f32, isOutput=False)
    W3 = nc.declare_dram_parameter("W3", [H2, 1], f32, isOutput=False)
    b3r = nc.declare_dram_parameter("b3r", [IPC, 1], f32, isOutput=False)
    out_c = nc.declare_dram_parameter("out_c", [IPC, N2], f32, isOutput=True)

    with tile.TileContext(nc) as tc:
        with (
            tc.tile_pool(name="const", bufs=1) as cpool,
            tc.tile_pool(name="work", bufs=3) as wpool,
            tc.tile_pool(name="psum", bufs=2, space="PSUM") as pp,
            tc.tile_pool(name="dram", bufs=1, space="DRAM") as dpool,
        ):
            # ---------- pooling (f2 first: feeds the AllGather critical path) ----
            ones_f32 = cpool.tile([128, 2], f32, tag="ones_f32")
            nc.vector.memset(ones_f32[:], 1.0 / L)
            ones_t = cpool.tile([128, 2], f32r, tag="ones")
            nc.vector.tensor_copy(ones_t[:], ones_f32[:])

            # pT[d, row] = mean_l f[row, l, d]; f32r matmuls (f tile is the
            # stationary operand) accumulate columns into persistent psum tiles
            NLD = 4  # load chunks per source
            pool_sb = {}
            for (name, src, nrow) in (("p2", f2c, JPC), ("p1", f1c, IPC)):
                fall = cpool.tile([128, nrow * D], f32r, tag=f"fall_{name}")
                rows_per = nrow // NLD
                for c in range(NLD):
                    nc.gpsimd.dma_start(
                        out=fall[:, c * rows_per * D:(c + 1) * rows_per * D]
                            .rearrange("l (r d) -> l r d", r=rows_per),
                        in_=src[c * rows_per:(c + 1) * rows_per]
                            .rearrange("r l d -> l r d"))
                ps_dc = [pp.tile([128, 2 * nrow], f32, tag="small",
                                 name=f"ps_{name}_{dc}")
                         for dc in range(DC)]
                for r in range(nrow):
                    for dc in range(DC):
                        nc.tensor.matmul(
                            ps_dc[dc][:, 2 * r:2 * r + 2],
                            fall[:, r * D + dc * 128:r * D + (dc + 1) * 128],
                            ones_t[:], start=True, stop=True)
                pT = cpool.tile([128, DC * nrow], f32r, tag=f"{name}T")
                for dc in range(DC):
                    nc.vector.tensor_copy(pT[:, dc * nrow:(dc + 1) * nrow],
                                          ps_dc[dc][:, 0::2])
                pool_sb[name] = pT
            p2T = pool_sb["p2"]
            p1Tr = pool_sb["p1"]

            # ---------- AllGather p2T ----------
            p2loc = dpool.tile([DC, 128, JPC], f32, tag="p2loc")
            p2glob = dpool.tile([N_CORES, DC, 128, JPC], f32, tag="p2glob")
            nc.sync.dma_start(
                out=p2loc[:].rearrange("dc d j -> d dc j"),
                in_=p2T[:].bitcast(f32).rearrange("d (dc j) -> d dc j", dc=DC))
            nc.gpsimd.collective_compute(
                "AllGather",
                mybir.AluOpType.bypass,
                ins=[p2loc[:].opt()],
                outs=[p2glob[:].opt()],
                replica_groups=[list(range(N_CORES))],
            )
            p2all = []
            for dc in range(DC):
                t = cpool.tile([128, N2], f32r, tag=f"p2all_{dc}")
                nc.gpsimd.dma_start(
                    out=t[:].rearrange("d (c j) -> d c j", c=N_CORES),
                    in_=p2glob[:, dc, :, :].rearrange("c d j -> d c j"))
                p2all.append(t)

            # ---------- constants / weights ----------
            b1T = cpool.tile([128, KCH], f32, tag="b1T")     # b1T[p, kc] = b1[kc*128+p]
            nc.sync.dma_start(out=b1T[:], in_=b1[:].rearrange("(c p) -> p c", p=128))
            b2T = cpool.tile([128, CB], f32, tag="b2T")
            nc.sync.dma_start(out=b2T[:], in_=b2[:].rearrange("(c p) -> p c", p=128))
            w3T = cpool.tile([128, CB], bf16, tag="w3T")
            nc.gpsimd.dma_start(out=w3T[:],
                                in_=W3[:].rearrange("(c p) o -> p (c o)", p=128))
            b3t = cpool.tile([IPC, 1], f32, tag="b3t")
            nc.sync.dma_start(out=b3t[:], in_=b3r[:])

            # W2 in bf16, one mega cast DMA; w2m[:, kc*H2 + c]
            w2m = cpool.tile([128, KCH * H2], bf16, tag="w2m")
            nc.gpsimd.dma_start(
                out=w2m[:].rearrange("p (kc c) -> p kc c", kc=KCH),
                in_=W2[:].rearrange("(kc p) c -> p kc c", p=128))

            # U = W1a + W1c, V = W1b - W1c  (f32r, layout [d, k'])
            w1t = []
            for r in range(6):
                t = cpool.tile([128, DENSE], f32, tag=f"w1_{r}")
                nc.sync.dma_start(out=t[:], in_=W1[r * 128:(r + 1) * 128, :])
                w1t.append(t)
            Ut, Vt = [], []
            for dc in range(DC):
                u = cpool.tile([128, DENSE], f32r, tag=f"u_{dc}")
                v = cpool.tile([128, DENSE], f32r, tag=f"v_{dc}")
                nc.vector.tensor_add(u[:], w1t[dc][:], w1t[4 + dc][:])
                nc.vector.tensor_sub(v[:], w1t[2 + dc][:], w1t[4 + dc][:])
                Ut.append(u)
                Vt.append(v)

            # ---------- A1T (+b1 -> B1T, i-major), A2T (j-major) ----------
            B1T = cpool.tile([128, IPC * KCH], bf16, tag="B1T")  # col = i*KCH + kc
            A2T = cpool.tile([128, KCH * N2], bf16, tag="A2T")   # col = kc*N2 + j
            for kb in range(KCH):
                pa = pp.tile([128, IPC], f32, tag="small")
                for dc in range(DC):
                    nc.tensor.matmul(pa[:], Ut[dc][:, kb * 128:(kb + 1) * 128],
                                     p1Tr[:, dc * IPC:(dc + 1) * IPC],
                                     start=(dc == 0), stop=(dc == DC - 1))
                nc.vector.tensor_scalar_add(B1T[:, kb::KCH], pa[:],
                                            b1T[:, kb:kb + 1])
            for kb in range(KCH):
                pa2 = pp.tile([128, N2], f32, tag="big")
                for dc in range(DC):
                    nc.tensor.matmul(pa2[:], Vt[dc][:, kb * 128:(kb + 1) * 128],
                                     p2all[dc][:], start=(dc == 0),
                                     stop=(dc == DC - 1))
                nc.vector.tensor_copy(A2T[:, kb * N2:(kb + 1) * N2], pa2[:])

            # ---------- main loop over i ----------
            outst = cpool.tile([IPC, N2], f32, tag="outst")
            for i in range(IPC):
                # x1pre = A2T + B1T[:, i-block] broadcast over j (2x bf16 DVE)
                x1pre = wpool.tile([128, KCH * N2], bf16, tag="x1pre")
                nc.vector.tensor_add(
                    x1pre[:].rearrange("p (k j) -> p k j", k=KCH),
                    A2T[:].rearrange("p (k j) -> p k j", k=KCH),
                    B1T[:, i * KCH:(i + 1) * KCH].broadcast_to((128, KCH, N2)))
                # x1 = gelu(x1pre) -> bf16; re-layout j-major -> kc-major so the
                # matmul rhs below is contiguous (strided ACT input)
                x1T = wpool.tile([128, KCH * N2], bf16, tag="x1T")
                nc.scalar.activation(x1T[:], x1pre[:], AF.Gelu)

                # layer 2: h2preT[cb] = sum_kc W2[kc,cb].T @ x1T[kc]
                ph2 = pp.tile([128, CB * N2], f32, tag="big")
                for cb in range(CB):
                    for kc in range(KCH):
                        nc.tensor.matmul(
                            ph2[:, cb * N2:(cb + 1) * N2],
                            w2m[:, kc * H2 + cb * 128:kc * H2 + (cb + 1) * 128],
                            x1T[:, kc * N2:(kc + 1) * N2],
                            start=(kc == 0), stop=(kc == KCH - 1))
                # h2b = ph2 + b2 (broadcast), then gelu -> bf16
                h2b = wpool.tile([128, CB * N2], f32, tag="h2b")
                nc.vector.tensor_add(
                    h2b[:].rearrange("p (c j) -> p c j", c=CB),
                    ph2[:].rearrange("p (c j) -> p c j", c=CB),
                    b2T[:].broadcast_to((128, CB, N2)))
                h2T = wpool.tile([128, CB * N2], bf16, tag="h2T")
                nc.scalar.activation(h2T[:], h2b[:], AF.Gelu)

                # layer 3: out_pre[1, N2] = sum_cb w3T[:, cb].T @ h2T[cb]
                pl3 = pp.tile([1, N2], f32, tag="small")
                for cb in range(CB):
                    nc.tensor.matmul(pl3[:], w3T[:, cb:cb + 1],
                                     h2T[:, cb * N2:(cb + 1) * N2],
                                     start=(cb == 0), stop=(cb == CB - 1))
                # collect row i (cross-partition move via small DMA)
                orow = wpool.tile([1, N2], f32, tag="orow")
                nc.vector.tensor_copy(orow[:], pl3[:])
                nc.sync.dma_start(out=outst[i:i + 1, :], in_=orow[:])

            # ---------- sigmoid + store ----------
            osg = cpool.tile([IPC, N2], f32, tag="osg")
            nc.scalar.activation(osg[:], outst[:], AF.Sigmoid, bias=b3t[:])
            nc.sync.dma_start(out=out_c[:], in_=osg[:])

    nc.finalize()
    return nc


def kernel(**inputs):
    from concourse.bass_utils import run_bass_kernel_spmd

    global _CACHED_NC
    f1 = np.ascontiguousarray(np.asarray(inputs["f1"], dtype=np.float32))
    f2 = np.ascontiguousarray(np.asarray(inputs["f2"], dtype=np.float32))
    W1 = np.ascontiguousarray(np.asarray(inputs["W1"], dtype=np.float32))
    b1 = np.asarray(inputs["b1"], dtype=np.float32)
    W2 = np.ascontiguousarray(np.asarray(inputs["W2"], dtype=np.float32))
    b2 = np.asarray(inputs["b2"], dtype=np.float32)
    W3 = np.ascontiguousarray(np.asarray(inputs["W3"], dtype=np.float32))
    b3 = np.asarray(inputs["b3"], dtype=np.float32)
    b3r = np.full((IPC, 1), b3.reshape(-1)[0], dtype=np.float32)

    if _CACHED_NC is None:
        _CACHED_NC = build_kernel()
    nc = _CACHED_NC

    in_maps = []
    for k in range(N_CORES):
        in_maps.append({
            "f1c": np.ascontiguousarray(f1[k * IPC:(k + 1) * IPC]),
            "f2c": np.ascontiguousarray(f2[k * JPC:(k + 1) * JPC]),
            "W1": W1, "b1": b1, "W2": W2, "b2": b2, "W3": W3, "b3r": b3r,
        })
    res = run_bass_kernel_spmd(nc, in_maps, core_ids=list(range(N_CORES)))
    out = np.concatenate([res.results[k]["out_c"] for k in range(N_CORES)],
                         axis=0)
    return out.astype(np.float32)
